# revision 12
# baseline (speedup 1.0000x reference)
"""Trainium2 Bass kernel for nn_BDLModel (gnn_message_passing).

Strategy (8 NeuronCores, SPMD):
  - Nodes sharded contiguously across cores (3750/core); within a core, nodes
    are assigned to 128-row dst tiles by balanced in-degree (LPT) to minimize
    the padded edge-chunk count CK; the output rows are inverse-permuted on
    the host.
  - Mean aggregation: the activation table is AllGathered to DRAM in fp8e4m3
    (values pre-scaled by 16; the 1/16 is folded into 1/deg), split into two
    half-tables so each collective fires as soon as its rows are written and
    overlaps compute. Per dst-tile, four dma_gathers (one per swdge queue,
    half-table split) pull all source rows ([128, CK, W] edge-major); per
    pair of 128-edge chunks a one-hot fp8 selection matrix S (iota vs
    per-edge dst-local ids) is the stationary operand of a DoubleRow fp8
    matmul: psum[d, :] += S^T @ Zgathered; 1/deg applied on PSUM evacuation.
  - SAGE MLPs run as fp8 DoubleRow matmuls (weights x32, activations x16,
    1/512 descale at evacuation); the MLP2 residual is added at evacuation.
  - Householder D=2 closed form: Q = [[c, s], [-s, c]], c=(a^2-1)/(1+a^2),
    s=2a/(1+a^2); only the 2::4 columns of enc_w2 are needed.
  - Self-z activations stay resident in SBUF; biases/residuals are preloaded
    into PSUM (Scalar/Vector) so matmul chains accumulate on top; PSUM
    evacuations and LayerNorm row-stats run on the Scalar engine.
"""

import math
import os
import sys
from dataclasses import dataclass

import numpy as np

for _p in ("/opt/trn_rl_repo", "/root/.axon_site/_ro/trn_rl_repo"):
    if os.path.isdir(_p) and _p not in sys.path:
        sys.path.insert(0, _p)

import ml_dtypes  # noqa: E402

BF16 = ml_dtypes.bfloat16

F8SCALE = 16.0


@dataclass(frozen=True)
class Cfg:
    N: int = 30000
    E: int = 480000
    HID: int = 256
    NB: int = 128
    D: int = 2
    NL: int = 2
    NSAGE: int = 5
    OUT: int = 5
    NC: int = 8
    EPS: float = 1e-5

    @property
    def SW(self):
        return self.D * self.D * self.NB

    @property
    def NLOC(self):
        return self.N // self.NC

    @property
    def NT(self):
        return (self.NLOC + 127) // 128

    @property
    def HLF(self):
        """Row boundary for the two per-layer table collectives. Asymmetric
        (70/30): the small B half keeps the end-of-layer collective flight
        short, since it head-of-line-blocks the next layer's gather emission
        on the Pool queue; the large A half still completes well before the
        next layer needs it."""
        return (7 * self.NLOC) // 10

    @property
    def HT(self):
        """First tile index whose rows complete the A half."""
        return (self.HLF + 127) // 128


CFG = Cfg()


# ---------------------------------------------------------------- host prep


W8SCALE = 32.0
F8NP = getattr(ml_dtypes, "float8_e4m3fn", None) or ml_dtypes.float8_e4m3


def _prep_rhs8(w):
    """fp8 weight panel, pre-scaled by W8SCALE; same [128, kc*M] layout."""
    k, m = w.shape
    kc = k // 128
    return np.ascontiguousarray(
        (w * W8SCALE).reshape(kc, 128, m).transpose(1, 0, 2).reshape(128, kc * m)
    ).astype(F8NP)


def _prep_rhs(w):
    """[K, M] -> [128, (K//128)*M] so slice kc -> [:, kc*M:(kc+1)*M] = W[kc]."""
    k, m = w.shape
    assert k % 128 == 0
    kc = k // 128
    return np.ascontiguousarray(
        w.reshape(kc, 128, m).transpose(1, 0, 2).reshape(128, kc * m)
    ).astype(BF16)


def _prep_bias(b):
    return np.ascontiguousarray(np.tile(np.asarray(b, np.float32).reshape(1, -1), (128, 1)))


def _balance_tiles(deg_local, nt):
    """LPT: assign nodes to nt tiles of <=128 nodes, balancing summed degree.
    Returns perm (tile-major node order)."""
    import heapq

    order = np.argsort(-deg_local, kind="stable")
    heap = [(0, t) for t in range(nt)]
    heapq.heapify(heap)
    counts = [0] * nt
    members = [[] for _ in range(nt)]
    for v in order:
        while True:
            load, t = heapq.heappop(heap)
            if counts[t] < 128:
                break
        members[t].append(v)
        counts[t] += 1
        if counts[t] < 128:
            heapq.heappush(heap, (load + int(deg_local[v]), t))
    return np.concatenate([np.asarray(m, np.int64) for m in members])


def _prep_edges(cfg: Cfg, edge_index):
    """Balanced tile assignment (tiles ordered by descending edge count so the
    shared program's per-iteration chunk maxima stay tight); edges partitioned
    by dst owner; exact per-tile chunk schedule."""
    src = np.asarray(edge_index[0], np.int64)
    dst = np.asarray(edge_index[1], np.int64)
    deg = np.bincount(dst, minlength=cfg.N).astype(np.float64)
    rdeg_full = (1.0 / np.maximum(deg, 1.0)).astype(np.float32) / F8SCALE

    NLOC, NT = cfg.NLOC, cfg.NT
    # per-core node order: LPT perm, then 128-blocks sorted by edge count
    # (partial block pinned last so rows_of(t) stays program-side).
    perms, poss = [], []
    for c in range(cfg.NC):
        dl = deg[c * NLOC : (c + 1) * NLOC]
        perm0 = _balance_tiles(dl, NT)
        pos0 = np.empty(NLOC, np.int64)
        pos0[perm0] = np.arange(NLOC)
        m = (dst >= c * NLOC) & (dst < (c + 1) * NLOC)
        blk = pos0[dst[m] - c * NLOC] // 128
        cnt = np.bincount(blk, minlength=NT)
        full = np.argsort(-cnt[: NT - 1], kind="stable")
        order = np.concatenate([full, [NT - 1]])
        perm = np.concatenate([perm0[b * 128 : b * 128 + 128] for b in order])
        pos = np.empty(NLOC, np.int64)
        pos[perm] = np.arange(NLOC)
        perms.append(perm)
        poss.append(pos)

    # table row of global node v: half-split layout. srcs with pos < HLF live
    # in table A at row c*HLF + pos; the rest in table B at c*HLF + (pos-HLF).
    HLF = cfg.HLF
    src_c = src // NLOC
    src_p = np.concatenate([poss[c][None, :] for c in range(cfg.NC)], axis=0)[
        src_c, src % NLOC
    ]
    in_a = src_p < HLF
    src_row = np.where(in_a, src_c * HLF + src_p, src_c * (NLOC - HLF) + src_p - HLF)
    assert src_row.max() < 32768

    per_core = []
    cnta_all = np.zeros((cfg.NC, NT), np.int64)
    cntb_all = np.zeros((cfg.NC, NT), np.int64)
    for c in range(cfg.NC):
        lo, hi = c * NLOC, (c + 1) * NLOC
        m = (dst >= lo) & (dst < hi)
        s_row = src_row[m]
        s_a = in_a[m]
        d_pos = poss[c][dst[m] - lo]
        # order by (dst tile, B-half flag) so each tile lists A edges then B
        order = np.lexsort((~s_a, d_pos // 128))
        s_row, d_pos, s_a = s_row[order], d_pos[order], s_a[order]
        tb = np.searchsorted(d_pos // 128, np.arange(NT + 1))
        for t in range(NT):
            seg = s_a[tb[t] : tb[t + 1]]
            cnta_all[c, t] = int(seg.sum())
            cntb_all[c, t] = len(seg) - cnta_all[c, t]
        per_core.append((s_row, d_pos, tb, cnta_all[c].copy(), cntb_all[c].copy()))

    # shared per-iteration chunk counts: max over cores, at least 1
    cka = tuple(max(1, int(math.ceil(cnta_all[:, t].max() / 128))) for t in range(NT))
    ckb = tuple(max(1, int(math.ceil(cntb_all[:, t].max() / 128))) for t in range(NT))
    toff = [0] * (NT + 1)
    for t in range(NT):
        toff[t + 1] = toff[t] + cka[t] + ckb[t]
    TC = toff[NT]

    outs = []
    for c in range(cfg.NC):
        s_row, d_pos, tb, cnta, cntb = per_core[c]
        idx16 = np.zeros((128, TC * 8), np.int16)
        ids = np.full((128, TC), 255.0, BF16)
        for t in range(NT):
            na, nb = int(cnta[t]), int(cntb[t])
            for (n, roff, coff) in ((na, tb[t], toff[t]), (nb, tb[t] + na, toff[t] + cka[t])):
                if n == 0:
                    continue
                i = np.arange(n)
                # gather order: unwrapped[i] = idx16[i%16, i//16] (replicated x8)
                col = coff * 8 + i // 16
                row = i % 16
                sv = s_row[roff : roff + n]
                for g in range(8):
                    idx16[row + 16 * g, col] = sv
                ids[i % 128, coff + i // 128] = (
                    d_pos[roff : roff + n] - t * 128
                ).astype(BF16)
        rdeg = np.ones((128, NT), np.float32) / F8SCALE
        rfull = rdeg_full[c * NLOC : (c + 1) * NLOC][perms[c]]
        for t in range(NT):
            r0 = t * 128
            nr = min(128, NLOC - r0)
            rdeg[:nr, t] = rfull[r0 : r0 + nr]
        outs.append(dict(idx16=idx16, ids=ids, rdeg=rdeg, rdeg16=rdeg * F8SCALE))
    return (cka, ckb), outs, perms


def _prep_inputs(cfg: Cfg, inputs):
    """Build the per-core in_maps. Returns ((CKA, CKB), in_maps, perms)."""
    f32 = np.float32
    f64 = np.float64
    x = np.asarray(inputs["x"], f32)
    CK, edge_outs, perms = _prep_edges(cfg, np.asarray(inputs["edge_index"]))

    g = lambda k: np.asarray(inputs[k], f32)

    # fold the struct-encoder output projection into the first encoder MLP
    so_w = np.asarray(inputs["se_out_w"], f64)
    so_b = np.asarray(inputs["se_out_b"], f64)
    w_f1, b_f1 = [], []
    for k in range(cfg.NL):
        e1w = np.asarray(inputs["enc_w1"], f64)[k]
        e1b = np.asarray(inputs["enc_b1"], f64)[k]
        w_f1.append((so_w @ e1w).astype(f32))
        b_f1.append((so_b @ e1w + e1b).astype(f32))

    shared = {
        "w_in_r": _prep_rhs(g("w_in")),
        "b_in_bc": _prep_bias(g("b_in")),
        "w_si_r": _prep_rhs(g("se_in_w")),
        "b_si_bc": _prep_bias(g("se_in_b")),
        "w_s1_r": np.concatenate([_prep_rhs8(g("sage_w1")[i]) for i in range(cfg.NSAGE)], axis=1),
        "b_s1_bc": np.concatenate(
            [_prep_bias(g("sage_b1")[i] * F8SCALE * W8SCALE) for i in range(cfg.NSAGE)], axis=1
        ).astype(BF16),
        "w_s2_r": np.concatenate([_prep_rhs8(g("sage_w2")[i]) for i in range(cfg.NSAGE)], axis=1),
        "b_s2_bc": np.concatenate(
            [_prep_bias(g("sage_b2")[i] * F8SCALE * W8SCALE) for i in range(cfg.NSAGE)], axis=1
        ).astype(BF16),
        "w_e1_r": np.concatenate([_prep_rhs(w_f1[k]) for k in range(cfg.NL)], axis=1),
        "b_e1_bc": np.concatenate([_prep_bias(b_f1[k]) for k in range(cfg.NL)], axis=1),
        "w_e2_r": np.concatenate(
            [_prep_rhs(np.ascontiguousarray(g("enc_w2")[k][:, 2::4])) for k in range(cfg.NL)], axis=1
        ),
        "b_e2_bc": np.concatenate([_prep_bias(g("enc_b2")[k][2::4]) for k in range(cfg.NL)], axis=1),
        "ln_g_bc": np.concatenate([_prep_bias(g("ln_g")[k]) for k in range(cfg.NL)], axis=1),
        "ln_b_bc": np.concatenate([_prep_bias(g("ln_b")[k]) for k in range(cfg.NL)], axis=1),
        "w_b1_r": np.concatenate([_prep_rhs(g("bdl_w1")[k]) for k in range(cfg.NL)], axis=1),
        "b_b1_bc": np.concatenate([_prep_bias(g("bdl_b1")[k]) for k in range(cfg.NL)], axis=1),
        "w_b2_r": np.concatenate([_prep_rhs(g("bdl_w2")[k]) for k in range(cfg.NL)], axis=1),
        "b_b2_bc": np.concatenate([_prep_bias(g("bdl_b2")[k]) for k in range(cfg.NL)], axis=1),
        "oln_g_bc": _prep_bias(g("out_ln_g")),
        "oln_b_bc": _prep_bias(g("out_ln_b")),
        "w_o_r": _prep_rhs(g("w_out")),
        "b_o_bc": _prep_bias(g("b_out")),
        "ident_b": np.eye(128, dtype=BF16),
        "iota_f": np.tile(np.arange(128), (128, 1)).astype(BF16),
    }

    in_maps = []
    for c in range(cfg.NC):
        m = dict(shared)
        m["x_c"] = np.ascontiguousarray(x[c * cfg.NLOC : (c + 1) * cfg.NLOC][perms[c]])
        m["idx16"] = edge_outs[c]["idx16"]
        m["ids_f"] = edge_outs[c]["ids"]
        m["rdeg"] = edge_outs[c]["rdeg"]
        m["rdeg16"] = edge_outs[c]["rdeg16"]
        in_maps.append(m)
    return CK, in_maps, perms


# ---------------------------------------------------------------- builder


def build_program(cfg: Cfg, cka: tuple, ckb: tuple, sage_bias_zero: bool = True):
    from concourse import bacc, mybir
    import concourse.tile as tile

    f32 = mybir.dt.float32
    bf16 = mybir.dt.bfloat16
    fp8 = mybir.dt.float8e4
    i16 = mybir.dt.int16
    ALU = mybir.AluOpType
    AX = mybir.AxisListType
    ACT = mybir.ActivationFunctionType

    NT, NLOC, HID, SW = cfg.NT, cfg.NLOC, cfg.HID, cfg.SW
    HLF, HT = cfg.HLF, cfg.HT
    toff = [0] * (NT + 1)
    for _t in range(NT):
        toff[_t + 1] = toff[_t] + cka[_t] + ckb[_t]
    TC = toff[NT]
    CKMAX = max(cka[_t] + ckb[_t] for _t in range(NT))

    nc = bacc.Bacc(
        "TRN2",
        target_bir_lowering=False,
        debug=False,
        enable_asserts=False,
        num_devices=cfg.NC,
        num_swdge_queues=4,
    )
    rg = [list(range(cfg.NC))]

    # ---- external I/O
    d_x = nc.dram_tensor("x_c", [NLOC, HID], f32, kind="ExternalInput").ap()
    d_idx = nc.dram_tensor("idx16", [128, TC * 8], i16, kind="ExternalInput").ap()
    d_ids = nc.dram_tensor("ids_f", [128, TC], bf16, kind="ExternalInput").ap()
    d_rdeg = nc.dram_tensor("rdeg", [128, NT], f32, kind="ExternalInput").ap()
    d_rdeg16 = nc.dram_tensor("rdeg16", [128, NT], f32, kind="ExternalInput").ap()

    def din(name, shape, dt):
        return nc.dram_tensor(name, shape, dt, kind="ExternalInput").ap()

    NS, NL = cfg.NSAGE, cfg.NL
    d_w_in = din("w_in_r", [128, 2 * HID], bf16)
    d_b_in = din("b_in_bc", [128, HID], f32)
    d_w_si = din("w_si_r", [128, 2 * SW], bf16)
    d_b_si = din("b_si_bc", [128, SW], f32)
    d_w_s1 = din("w_s1_r", [128, NS * 8 * SW], fp8)
    d_b_s1 = din("b_s1_bc", [128, NS * SW], bf16)
    d_w_s2 = din("w_s2_r", [128, NS * 4 * SW], fp8)
    d_b_s2 = din("b_s2_bc", [128, NS * SW], bf16)
    d_w_e1 = din("w_e1_r", [128, NL * 4 * SW], bf16)
    d_b_e1 = din("b_e1_bc", [128, NL * SW], f32)
    d_w_e2 = din("w_e2_r", [128, NL * 4 * 128], bf16)
    d_b_e2 = din("b_e2_bc", [128, NL * 128], f32)
    d_ln_g = din("ln_g_bc", [128, NL * HID], f32)
    d_ln_b = din("ln_b_bc", [128, NL * HID], f32)
    d_w_b1 = din("w_b1_r", [128, NL * 4 * HID], bf16)
    d_b_b1 = din("b_b1_bc", [128, NL * HID], f32)
    d_w_b2 = din("w_b2_r", [128, NL * 2 * HID], bf16)
    d_b_b2 = din("b_b2_bc", [128, NL * HID], f32)
    d_oln_g = din("oln_g_bc", [128, HID], f32)
    d_oln_b = din("oln_b_bc", [128, HID], f32)
    d_w_o = din("w_o_r", [128, 2 * cfg.OUT], bf16)
    d_b_o = din("b_o_bc", [128, cfg.OUT], f32)
    d_identb = din("ident_b", [128, 128], bf16)
    d_iota = din("iota_f", [128, 128], bf16)

    d_out = nc.dram_tensor("out", [NLOC, cfg.OUT], f32, kind="ExternalOutput").ap()

    with tile.TileContext(nc) as tc:
        from contextlib import ExitStack

        ctx = ExitStack()
        pers = ctx.enter_context(tc.tile_pool(name="pers", bufs=1))
        wout = ctx.enter_context(tc.tile_pool(name="wout", bufs=1))
        wsage = ctx.enter_context(tc.tile_pool(name="wsage", bufs=1))
        work = ctx.enter_context(tc.tile_pool(name="work", bufs=2))
        small = ctx.enter_context(tc.tile_pool(name="small", bufs=2))
        spool = ctx.enter_context(tc.tile_pool(name="spool", bufs=2))
        zgp = ctx.enter_context(tc.tile_pool(name="zgp", bufs=2))
        dram = ctx.enter_context(tc.tile_pool(name="dram", bufs=2, space="DRAM"))
        psum = ctx.enter_context(tc.tile_pool(name="psum", bufs=2, space="PSUM"))

        # ---- persistent SBUF residents
        identb = pers.tile([128, 128], bf16, name="identb")
        iota = pers.tile([128, 128], bf16, name="iota")
        rdeg = pers.tile([128, NT], f32, name="rdegs")
        rdeg16 = pers.tile([128, NT], f32, name="rdeg16s")
        ids = pers.tile([128, TC], bf16, name="idss")
        idx = pers.tile([128, TC * 8], i16, name="idxs")
        nc.sync.dma_start(out=identb[:], in_=d_identb[:])
        nc.sync.dma_start(out=iota[:], in_=d_iota[:])
        nc.sync.dma_start(out=rdeg[:], in_=d_rdeg[:])
        nc.sync.dma_start(out=rdeg16[:], in_=d_rdeg16[:])
        nc.sync.dma_start(out=ids[:], in_=d_ids[:])
        nc.sync.dma_start(out=idx[:], in_=d_idx[:])

        ones1 = pers.tile([1, 128], bf16, name="ones1")
        nc.gpsimd.memset(ones1[:], 1.0)
        h_t = [pers.tile([128, HID], f32, name=f"h{t}") for t in range(NT)]
        z_t = [pers.tile([128, SW], bf16, name=f"z{t}") for t in range(NT)]
        c_t = [pers.tile([128, 128], bf16, name=f"rc{t}") for t in range(NT)]
        s_t = [pers.tile([128, 128], bf16, name=f"rs{t}") for t in range(NT)]

        def load_w(pool, name, src, cols, dt):
            t = pool.tile([128, cols], dt, tag=name, name=name)
            nc.sync.dma_start(out=t[:], in_=src)
            return t

        def load_w3(pool, name, src, kchunks, m, dt):
            """Weight panel as a native 3-D tile so DoubleRow rhs slices match
            the zg access-pattern shape exactly."""
            t = pool.tile([128, kchunks, m], dt, tag=name, name=name)
            nc.sync.dma_start(out=t[:].rearrange("p a b -> p (a b)"), in_=src)
            return t

        def dram_tile(name, shape, dt, shared=False):
            return dram.tile(
                shape, dt, tag=name, name=name, addr_space="Shared" if shared else "Local"
            )

        def rows_of(t):
            return min(128, NLOC - t * 128)

        # ---------- helpers ----------
        def transpose_into(dst, src_ap, nchunks, out_scale=None):
            """dst[:, kc*128:(kc+1)*128] = src[:, kc*128:(kc+1)*128]^T (bf16).
            All chunks land in one PSUM bank (start zero-fills the region),
            evacuated with a single vector copy."""
            tp = psum.tile([128, 4 * 128], bf16, tag="tr", name="trb")
            for kc in range(nchunks):
                nc.tensor.matmul(
                    tp[:, kc * 128 : (kc + 1) * 128],
                    lhsT=src_ap[:, kc * 128 : (kc + 1) * 128],
                    rhs=identb[:],
                    is_transpose=True,
                    start=(kc == 0),
                    stop=(kc == nchunks - 1),
                )
            if out_scale is None:
                nc.vector.tensor_copy(out=dst[:, : nchunks * 128], in_=tp[:, : nchunks * 128])
            else:
                nc.scalar.mul(
                    out=dst[:, : nchunks * 128], in_=tp[:, : nchunks * 128], mul=out_scale
                )

        def mm_acc(ps_ap, lhsT_tile, rhs_tile, kcs, m, rhs_block, preloaded=False):
            """ps (+)= sum_kc lhsT[:, kc]^T @ rhs[:, kc-block] (node-major out)."""
            for kc in range(kcs):
                nc.tensor.matmul(
                    ps_ap,
                    lhsT=lhsT_tile[:, kc * 128 : (kc + 1) * 128],
                    rhs=rhs_tile[:, kc * rhs_block + m[0] : kc * rhs_block + m[1]],
                    start=(kc == 0 and not preloaded),
                    stop=(kc == kcs - 1),
                    skip_group_check=preloaded,
                )

        def mm_dr(ps_ap, lhsT8, rhs8_3d, kpairs, preloaded=False):
            """ps (+)= paired fp8 DoubleRow accumulation over 2*kpairs chunks.
            rhs8_3d is a [128, 2*kpairs, M] tile; native slices keep the
            dual-pump stream rate."""
            for p in range(kpairs):
                nc.tensor.matmul(
                    ps_ap,
                    lhsT=lhsT8[:, 2 * p * 128 : (2 * p + 2) * 128].rearrange(
                        "q (two m) -> q two m", two=2
                    ),
                    rhs=rhs8_3d[:, 2 * p : 2 * p + 2, :],
                    start=(p == 0 and not preloaded),
                    stop=(p == kpairs - 1),
                    perf_mode=mybir.MatmulPerfMode.DoubleRow,
                    skip_group_check=preloaded,
                )

        def emit_ln(h_ap, g_bc_ap, b_bc_ap, out_ap, w):
            """LayerNorm with row stats on the Vector engine (bn_stats) and a
            table-free Newton rsqrt (avoids Sqrt activation-table churn)."""
            i32 = mybir.dt.int32
            st6 = small.tile([128, 6], f32, tag="ln1", name="ln1")
            nc.vector.bn_stats(out=st6[:], in_=h_ap)
            mv = small.tile([128, 2], f32, tag="ln2", name="ln2")
            nc.vector.bn_aggr(out=mv[:], in_=st6[:])
            vm = small.tile([128, 1], f32, tag="ln3", name="ln3")
            nc.vector.tensor_scalar(vm[:], mv[:, 1:2], 1.0, cfg.EPS, ALU.mult, ALU.add)
            y0i = small.tile([128, 1], i32, tag="ln4", name="ln4")
            nc.vector.tensor_scalar(y0i[:], vm[:].bitcast(i32), 1, None, ALU.logical_shift_right)
            nc.vector.tensor_scalar(y0i[:], y0i[:], -1, 0x5F3759DF, ALU.mult, ALU.add)
            rs = small.tile([128, 1], f32, tag="ln5", name="ln5")
            vy = small.tile([128, 1], f32, tag="ln6", name="ln6")
            ycur = y0i[:].bitcast(f32)
            for _ in range(2):
                nc.vector.tensor_tensor(out=vy[:], in0=vm[:], in1=ycur, op=ALU.mult)
                nc.vector.tensor_tensor(out=vy[:], in0=vy[:], in1=ycur, op=ALU.mult)
                nc.vector.tensor_scalar(vy[:], vy[:], -0.5, 1.5, ALU.mult, ALU.add)
                nc.vector.tensor_tensor(out=rs[:], in0=ycur, in1=vy[:], op=ALU.mult)
                ycur = rs[:]
            nmrs = small.tile([128, 1], f32, tag="ln7", name="ln7")
            nc.vector.scalar_tensor_tensor(
                out=nmrs[:], in0=mv[:, 0:1], scalar=-1.0, in1=rs[:],
                op0=ALU.mult, op1=ALU.mult,
            )
            hn0 = work.tile([128, w], bf16, tag="lnhn0", name="lnhn0")
            nc.scalar.activation(
                out=hn0[:], in_=h_ap, func=ACT.Identity, bias=nmrs[:], scale=rs[:]
            )
            hn1 = work.tile([128, w], bf16, tag="lnhn1", name="lnhn1")
            nc.vector.tensor_tensor(out=hn1[:], in0=hn0[:], in1=g_bc_ap, op=ALU.mult)
            nc.vector.tensor_tensor(out=out_ap, in0=hn1[:], in1=b_bc_ap, op=ALU.add)

        def emit_agg(tabA_ap, tabB_ap, t, width, ps_ap):
            """Gather + one-hot matmul segment sum for dst tile t into psum.
            A-half chunks (queues 0/1) read tabA; B-half (queues 2/3) tabB.
            Exact per-tile chunk counts."""
            CKA_t, CKB_t = cka[t], ckb[t]
            CK_t = CKA_t + CKB_t
            T0 = toff[t]
            zg = zgp.tile([128, CKMAX, width], fp8, tag="zg", name="zg", bufs=5)
            ca2, cb2 = (CKA_t + 1) // 2, (CKB_t + 1) // 2
            calls = (
                (0, 0, ca2, tabA_ap),
                (1, ca2, CKA_t, tabA_ap),
                (2, CKA_t, CKA_t + cb2, tabB_ap),
                (3, CKA_t + cb2, CK_t, tabB_ap),
            )
            for qi, c0, c1, tab in calls:
                if c1 > c0:
                    nc.gpsimd.dma_gather(
                        out_ap=zg[:, c0:c1, :],
                        in_ap=tab,
                        idxs_ap=idx[:, T0 * 8 + c0 * 8 : T0 * 8 + c1 * 8],
                        num_idxs=(c1 - c0) * 128,
                        num_idxs_reg=(c1 - c0) * 128,
                        elem_size=width,
                        single_packet=True,
                        queue_num=qi,
                    )
            Sall = spool.tile([128, CKMAX * 128], fp8, tag="S", name="S", bufs=4)
            iota_bc = iota[:].rearrange("p (o f) -> p o f", o=1).to_broadcast([128, CK_t, 128])
            ids_bc = (
                ids[:, T0 : T0 + CK_t]
                .rearrange("p (c o) -> p c o", o=1)
                .to_broadcast([128, CK_t, 128])
            )
            nc.vector.tensor_tensor(
                out=Sall[:, : CK_t * 128], in0=iota_bc, in1=ids_bc, op=ALU.is_equal
            )
            npair = CK_t // 2
            for p in range(npair):
                nc.tensor.matmul(
                    ps_ap,
                    lhsT=Sall[:, 2 * p * 128 : (2 * p + 2) * 128].rearrange(
                        "q (two m) -> q two m", two=2
                    ),
                    rhs=zg[:, 2 * p : 2 * p + 2, :],
                    start=(p == 0),
                    stop=(p == npair - 1 and CK_t % 2 == 0),
                    perf_mode=mybir.MatmulPerfMode.DoubleRow,
                )
            if CK_t % 2:
                nc.tensor.matmul(
                    ps_ap,
                    lhsT=Sall[:, (CK_t - 1) * 128 : CK_t * 128],
                    rhs=zg[:, CK_t - 1, :],
                    start=(CK_t == 1),
                    stop=True,
                )

        def fire_halves(t, loc_ap, tabA_ap, tabB_ap):
            """Issue the half-table AllGathers as their rows complete."""
            if t == HT - 1:
                nc.gpsimd.collective_compute(
                    "AllGather", ALU.bypass, replica_groups=rg,
                    ins=[loc_ap[0:HLF, :]], outs=[tabA_ap],
                )
            elif t == NT - 1:
                nc.gpsimd.collective_compute(
                    "AllGather", ALU.bypass, replica_groups=rg,
                    ins=[loc_ap[HLF:NLOC, :]], outs=[tabB_ap],
                )

        def emit_z0(t, w_si_sb, b_si_sb, loc8_ap, tabA_ap, tabB_ap, hb16):
            """z0 = gelu(h@w_si+b_si) -> z_t[t] (bf16) + fp8 table row write."""
            nr = rows_of(t)
            hT = work.tile([128, 2 * 128], bf16, tag="hT", name="hT")
            transpose_into(hT, hb16[:], 2)
            zp = psum.tile([128, SW], f32, tag="mlp", name="zp", bufs=3)
            nc.scalar.copy(out=zp[:], in_=b_si_sb[:])
            mm_acc(zp[:], hT, w_si_sb, 2, (0, SW), SW, preloaded=True)
            nc.scalar.activation(out=z_t[t][:], in_=zp[:], func=ACT.Gelu)
            z8 = work.tile([128, SW], fp8, tag="z8", name="z8", bufs=3)
            nc.vector.tensor_scalar(z8[:], z_t[t][:], F8SCALE, None, ALU.mult)
            nc.sync.dma_start(out=loc8_ap[t * 128 : t * 128 + nr, :], in_=z8[:nr, :])
            fire_halves(t, loc8_ap, tabA_ap, tabB_ap)

        # ================= phase 0: h0 = gelu(x @ w_in + b_in), z0 =================
        w_in_sb = load_w(wout, "w_in", d_w_in[:], 2 * HID, bf16)
        b_in_sb = load_w(wout, "b_in", d_b_in[:], HID, f32)
        w_si_sb = load_w(wout, "w_si", d_w_si[:], 2 * SW, bf16)
        b_si_sb = load_w(wout, "b_si", d_b_si[:], SW, f32)

        loc8 = dram_tile("loc8", [NLOC, SW], fp8)
        tabA = dram_tile("tabA", [cfg.NC * HLF, SW], fp8, shared=True)
        tabB = dram_tile("tabB", [cfg.NC * (NLOC - HLF), SW], fp8, shared=True)
        for t in range(NT):
            nr = rows_of(t)
            xt = work.tile([128, HID], f32, tag="lnjunk", name="xt")
            if nr < 128:
                nc.gpsimd.memset(xt[:], 0.0)
            nc.sync.dma_start(out=xt[:nr, :], in_=d_x[t * 128 : t * 128 + nr, :])
            xb = work.tile([128, HID], bf16, tag="hb16", name="xb")
            nc.vector.tensor_copy(out=xb[:], in_=xt[:])
            xT = work.tile([128, 2 * 128], bf16, tag="xT", name="xT")
            transpose_into(xT, xb[:], 2)
            hp = psum.tile([128, HID], f32, tag="mlp", name="hp", bufs=3)
            nc.scalar.copy(out=hp[:], in_=b_in_sb[:])
            mm_acc(hp[:], xT, w_in_sb, 2, (0, HID), HID, preloaded=True)
            nc.scalar.activation(out=h_t[t][:], in_=hp[:], func=ACT.Gelu)
            hb16 = work.tile([128, HID], bf16, tag="hb16", name="hb16")
            nc.vector.tensor_copy(out=hb16[:], in_=h_t[t][:])
            emit_z0(t, w_si_sb, b_si_sb, loc8, tabA, tabB, hb16)

        # ================= outer layers =================
        for k in range(NL):
            # ---- per-outer weights
            w_e1_sb = load_w(wout, "w_e1", d_w_e1[:, k * 4 * SW : (k + 1) * 4 * SW], 4 * SW, bf16)
            b_e1_sb = load_w(wout, "b_e1", d_b_e1[:, k * SW : (k + 1) * SW], SW, f32)
            w_e2_sb = load_w(wout, "w_e2", d_w_e2[:, k * 4 * 128 : (k + 1) * 4 * 128], 4 * 128, bf16)
            b_e2_sb = load_w(wout, "b_e2", d_b_e2[:, k * 128 : (k + 1) * 128], 128, f32)
            ln_g_sb = load_w(wout, "ln_g", d_ln_g[:, k * HID : (k + 1) * HID], HID, f32)
            ln_b_sb = load_w(wout, "ln_b", d_ln_b[:, k * HID : (k + 1) * HID], HID, f32)
            w_b1_sb = load_w(wout, "w_b1", d_w_b1[:, k * 4 * HID : (k + 1) * 4 * HID], 4 * HID, bf16)
            b_b1_sb = load_w(wout, "b_b1", d_b_b1[:, k * HID : (k + 1) * HID], HID, f32)
            w_b2_sb = load_w(wout, "w_b2", d_w_b2[:, k * 2 * HID : (k + 1) * 2 * HID], 2 * HID, bf16)
            b_b2_sb = load_w(wout, "b_b2", d_b_b2[:, k * HID : (k + 1) * HID], HID, f32)

            tabA_prev, tabB_prev = tabA, tabB
            locy8 = dram_tile("locy8", [NLOC, HID], fp8)
            ytabA = dram_tile("ytabA", [cfg.NC * HLF, HID], fp8, shared=True)
            ytabB = dram_tile("ytabB", [cfg.NC * (NLOC - HLF), HID], fp8, shared=True)

            # ---- SAGE layers
            for i in range(cfg.NSAGE):
                w1_sb = load_w3(wsage, "w1", d_w_s1[:, i * 8 * SW : (i + 1) * 8 * SW], 8, SW, fp8)
                b1_sb = load_w(wsage, "b1", d_b_s1[:, i * SW : (i + 1) * SW], SW, bf16)
                w2_sb = load_w3(wsage, "w2", d_w_s2[:, i * 4 * SW : (i + 1) * 4 * SW], 4, SW, fp8)
                b2_sb = load_w(wsage, "b2", d_b_s2[:, i * SW : (i + 1) * SW], SW, bf16)
                last = i == cfg.NSAGE - 1
                if not last:
                    loc8_cur = dram_tile("loc8", [NLOC, SW], fp8)
                    tabA_cur = dram_tile("tabA", [cfg.NC * HLF, SW], fp8, shared=True)
                    tabB_cur = dram_tile("tabB", [cfg.NC * (NLOC - HLF), SW], fp8, shared=True)

                for t in range(NT):
                    nr = rows_of(t)
                    # aggregation from previous table
                    aps = psum.tile([128, SW], f32, tag="agg", name="aps", bufs=3)
                    emit_agg(tabA_prev[:], tabB_prev[:], t, SW, aps[:])
                    m_sb = work.tile([128, SW], bf16, tag="msb", name="msb", bufs=3)
                    nc.scalar.mul(out=m_sb[:], in_=aps[:], mul=rdeg16[:, t : t + 1])
                    # zc^T = [z*16 | m*16]^T in fp8 (m_sb already x16 scaled)
                    zcT8 = work.tile([128, 8 * 128], fp8, tag="zcT", name="zcT8", bufs=3)
                    transpose_into(zcT8[:, : 4 * 128], z_t[t][:], 4, out_scale=F8SCALE)
                    transpose_into(zcT8[:, 4 * 128 : 8 * 128], m_sb[:], 4, out_scale=1.0)
                    # MLP1 (fp8 DoubleRow; psum holds 512x the true values)
                    p1p = psum.tile([128, SW], f32, tag="mlp", name="p1p", bufs=3)
                    if not sage_bias_zero:
                        nc.tensor.matmul(
                            p1p[:], lhsT=ones1[0:1, :], rhs=b1_sb[0:1, :], start=True, stop=False
                        )
                    mm_dr(p1p[:], zcT8, w1_sb, 4, preloaded=not sage_bias_zero)
                    p1 = work.tile([128, SW], bf16, tag="p1", name="p1", bufs=3)
                    nc.scalar.activation(out=p1[:], in_=p1p[:], func=ACT.Gelu, scale=1.0 / 512.0)
                    p1T8 = work.tile([128, 4 * 128], fp8, tag="p1T", name="p1T8", bufs=3)
                    transpose_into(p1T8, p1[:], 4, out_scale=F8SCALE)
                    # MLP2 (psum holds 512x); residual added at evacuation
                    p2p = psum.tile([128, SW], f32, tag="mlp", name="p2p", bufs=3)
                    if not sage_bias_zero:
                        nc.tensor.matmul(
                            p2p[:], lhsT=ones1[0:1, :], rhs=b2_sb[0:1, :], start=True, stop=False
                        )
                    mm_dr(p2p[:], p1T8, w2_sb, 2, preloaded=not sage_bias_zero)
                    nc.vector.scalar_tensor_tensor(
                        out=z_t[t][:], in0=p2p[:], scalar=1.0 / 512.0, in1=z_t[t][:],
                        op0=ALU.mult, op1=ALU.add,
                    )
                    if not last:
                        z8 = work.tile([128, SW], fp8, tag="z8", name="z8", bufs=3)
                        nc.vector.tensor_scalar(z8[:], z_t[t][:], F8SCALE, None, ALU.mult)
                        nc.sync.dma_start(
                            out=loc8_cur[t * 128 : t * 128 + nr, :], in_=z8[:nr, :]
                        )
                        fire_halves(t, loc8_cur, tabA_cur, tabB_cur)
                        continue

                    # ---- fused: enc path -> rotation coefs; LN(h) -> y -> locy8
                    z5T = work.tile([128, 4 * 128], bf16, tag="p1T", name="z5T", bufs=3)
                    transpose_into(z5T, z_t[t][:], 4)
                    gp = psum.tile([128, SW], f32, tag="mlp", name="gp", bufs=3)
                    nc.scalar.copy(out=gp[:], in_=b_e1_sb[:])
                    mm_acc(gp[:], z5T, w_e1_sb, 4, (0, SW), SW, preloaded=True)
                    gact = work.tile([128, SW], bf16, tag="p1", name="gact", bufs=3)
                    nc.scalar.activation(out=gact[:], in_=gp[:], func=ACT.Gelu)
                    gT = work.tile([128, 4 * 128], bf16, tag="p1T", name="gT", bufs=3)
                    transpose_into(gT, gact[:], 4)
                    ap_ = psum.tile([128, 128], f32, tag="agg", name="ap_", bufs=3)
                    nc.scalar.copy(out=ap_[:], in_=b_e2_sb[:])
                    mm_acc(ap_[:], gT, w_e2_sb, 4, (0, 128), 128, preloaded=True)
                    a_sb = work.tile([128, 128], f32, tag="a0", name="a_sb")
                    nc.scalar.copy(out=a_sb[:], in_=ap_[:])
                    a2 = work.tile([128, 128], f32, tag="a1", name="a2")
                    nc.vector.tensor_tensor(out=a2[:], in0=a_sb[:], in1=a_sb[:], op=ALU.mult)
                    rinv = work.tile([128, 128], f32, tag="a2t", name="rinv")
                    nc.vector.tensor_scalar(rinv[:], a2[:], 1.0, None, ALU.add)
                    nc.vector.reciprocal(out=rinv[:], in_=rinv[:])
                    nc.vector.tensor_scalar(a2[:], a2[:], -1.0, None, ALU.add)
                    nc.vector.tensor_tensor(out=c_t[t][:], in0=a2[:], in1=rinv[:], op=ALU.mult)
                    nc.vector.tensor_scalar(a_sb[:], a_sb[:], 2.0, None, ALU.mult)
                    nc.vector.tensor_tensor(out=s_t[t][:], in0=a_sb[:], in1=rinv[:], op=ALU.mult)

                    # LN(h) -> hn; y = rot(hn); y8 = y*16 fp8
                    hn = work.tile([128, HID], bf16, tag="hn", name="hn")
                    emit_ln(h_t[t][:], ln_g_sb[:], ln_b_sb[:], hn[:], HID)
                    hn_ev = hn[:, 0:HID:2]
                    hn_od = hn[:, 1:HID:2]
                    y = work.tile([128, HID], bf16, tag="y", name="y")
                    t0 = work.tile([128, 128], bf16, tag="r0", name="t0")
                    t1 = work.tile([128, 128], bf16, tag="r1", name="t1")
                    nc.vector.tensor_tensor(out=t0[:], in0=c_t[t][:], in1=hn_ev, op=ALU.mult)
                    nc.vector.tensor_tensor(out=t1[:], in0=s_t[t][:], in1=hn_od, op=ALU.mult)
                    nc.vector.tensor_tensor(out=y[:, 0:HID:2], in0=t0[:], in1=t1[:], op=ALU.add)
                    nc.vector.tensor_tensor(out=t0[:], in0=c_t[t][:], in1=hn_od, op=ALU.mult)
                    nc.vector.tensor_tensor(out=t1[:], in0=s_t[t][:], in1=hn_ev, op=ALU.mult)
                    nc.vector.tensor_tensor(
                        out=y[:, 1:HID:2], in0=t0[:], in1=t1[:], op=ALU.subtract
                    )
                    y8 = work.tile([128, HID], fp8, tag="y8", name="y8")
                    nc.vector.tensor_scalar(y8[:], y[:], F8SCALE, None, ALU.mult)
                    nc.sync.dma_start(
                        out=locy8[t * 128 : t * 128 + nr, :], in_=y8[:nr, :]
                    )
                    fire_halves(t, locy8, ytabA, ytabB)

                if not last:
                    tabA_prev, tabB_prev = tabA_cur, tabB_cur
                    loc8 = loc8_cur

            # ---- BDL message + MLP, h update (+ fused z0 of next layer / output)
            if k + 1 < NL:
                loc8_nxt = dram_tile("loc8", [NLOC, SW], fp8)
                tabA_nxt = dram_tile("tabA", [cfg.NC * HLF, SW], fp8, shared=True)
                tabB_nxt = dram_tile("tabB", [cfg.NC * (NLOC - HLF), SW], fp8, shared=True)
            else:
                oln_g_sb = load_w(wout, "oln_g", d_oln_g[:], HID, f32)
                oln_b_sb = load_w(wout, "oln_b", d_oln_b[:], HID, f32)
                w_o_sb = load_w(wout, "w_o", d_w_o[:], 2 * cfg.OUT, bf16)
                b_o_sb = load_w(wout, "b_o", d_b_o[:], cfg.OUT, f32)

            for t in range(NT):
                nr = rows_of(t)
                yps = psum.tile([128, HID], f32, tag="agg", name="yps", bufs=3)
                emit_agg(ytabA[:], ytabB[:], t, HID, yps[:])
                # hn first (independent of the aggregate)
                hn = work.tile([128, HID], bf16, tag="hn", name="hnb")
                emit_ln(h_t[t][:], ln_g_sb[:], ln_b_sb[:], hn[:], HID)
                ga = work.tile([128, HID], bf16, tag="ga", name="ga")
                nc.vector.tensor_scalar(ga[:], yps[:], rdeg[:, t : t + 1], None, ALU.mult)
                g_ev = ga[:, 0:HID:2]
                g_od = ga[:, 1:HID:2]
                msg = work.tile([128, HID], bf16, tag="msg", name="msg")
                t0 = work.tile([128, 128], bf16, tag="r0", name="t0b")
                t1 = work.tile([128, 128], bf16, tag="r1", name="t1b")
                nc.vector.tensor_tensor(out=t0[:], in0=c_t[t][:], in1=g_ev, op=ALU.mult)
                nc.vector.tensor_tensor(out=t1[:], in0=s_t[t][:], in1=g_od, op=ALU.mult)
                nc.vector.tensor_tensor(
                    out=msg[:, 0:HID:2], in0=t0[:], in1=t1[:], op=ALU.subtract
                )
                nc.vector.tensor_tensor(out=t0[:], in0=s_t[t][:], in1=g_ev, op=ALU.mult)
                nc.vector.tensor_tensor(out=t1[:], in0=c_t[t][:], in1=g_od, op=ALU.mult)
                nc.vector.tensor_tensor(
                    out=msg[:, 1:HID:2], in0=t0[:], in1=t1[:], op=ALU.add
                )
                hcT = work.tile([128, 4 * 128], bf16, tag="hcT", name="hcT")
                tp4 = psum.tile([128, 4 * 128], bf16, tag="tr", name="trh")
                for j, srcap in enumerate([hn[:, 0:128], hn[:, 128:256], msg[:, 0:128], msg[:, 128:256]]):
                    nc.tensor.matmul(
                        tp4[:, j * 128 : (j + 1) * 128], lhsT=srcap, rhs=identb[:],
                        is_transpose=True, start=(j == 0), stop=(j == 3),
                    )
                nc.vector.tensor_copy(out=hcT[:], in_=tp4[:])
                bp = psum.tile([128, HID], f32, tag="mlp", name="bp", bufs=3)
                nc.scalar.copy(out=bp[:], in_=b_b1_sb[:])
                mm_acc(bp[:], hcT, w_b1_sb, 4, (0, HID), HID, preloaded=True)
                tb = work.tile([128, HID], bf16, tag="tb", name="tb")
                nc.scalar.activation(out=tb[:], in_=bp[:], func=ACT.Gelu)
                tbT = work.tile([128, 2 * 128], bf16, tag="tbT", name="tbT")
                transpose_into(tbT, tb[:], 2)
                b2p = psum.tile([128, HID], f32, tag="mlp", name="b2p", bufs=3)
                nc.vector.tensor_tensor(out=b2p[:], in0=h_t[t][:], in1=b_b2_sb[:], op=ALU.add)
                mm_acc(b2p[:], tbT, w_b2_sb, 2, (0, HID), HID, preloaded=True)
                nc.vector.tensor_copy(out=h_t[t][:], in_=b2p[:])

                if k + 1 < NL:
                    hb16 = work.tile([128, HID], bf16, tag="hb16", name="hb16b")
                    nc.vector.tensor_copy(out=hb16[:], in_=h_t[t][:])
                    emit_z0(t, w_si_sb, b_si_sb, loc8_nxt, tabA_nxt, tabB_nxt, hb16)
                else:
                    hnf = work.tile([128, HID], bf16, tag="hn", name="hnf")
                    emit_ln(h_t[t][:], oln_g_sb[:], oln_b_sb[:], hnf[:], HID)
                    hnfT = work.tile([128, 2 * 128], bf16, tag="tbT", name="hnfT")
                    transpose_into(hnfT, hnf[:], 2)
                    op_ = psum.tile([128, cfg.OUT], f32, tag="mlp", name="op_", bufs=3)
                    nc.scalar.copy(out=op_[:], in_=b_o_sb[:])
                    mm_acc(op_[:], hnfT, w_o_sb, 2, (0, cfg.OUT), cfg.OUT, preloaded=True)
                    ot = work.tile([128, cfg.OUT], f32, tag="ot", name="ot")
                    nc.scalar.copy(out=ot[:], in_=op_[:])
                    nc.sync.dma_start(out=d_out[t * 128 : t * 128 + nr, :], in_=ot[:nr, :])

            if k + 1 < NL:
                tabA, tabB = tabA_nxt, tabB_nxt
                loc8 = loc8_nxt

        ctx.close()

    nc.compile()
    return nc


# ---------------------------------------------------------------- runner

_CACHE = {}


def _get_program(cfg: Cfg, cka: tuple, ckb: tuple, sage_bias_zero: bool):
    key = (cfg, cka, ckb, sage_bias_zero)
    if key not in _CACHE:
        _CACHE[key] = build_program(cfg, cka, ckb, sage_bias_zero)
    return _CACHE[key]


def run(inputs, cfg: Cfg = CFG, trace: bool = False):
    from concourse import bass_utils

    (cka, ckb), in_maps, perms = _prep_inputs(cfg, inputs)
    sbz = not (
        np.asarray(inputs["sage_b1"]).any() or np.asarray(inputs["sage_b2"]).any()
    )
    nc = _get_program(cfg, cka, ckb, sbz)
    res = bass_utils.run_bass_kernel_spmd(
        nc, in_maps, core_ids=list(range(cfg.NC)), trace=trace
    )
    out = np.empty((cfg.N, cfg.OUT), np.float32)
    for c in range(cfg.NC):
        out[c * cfg.NLOC + perms[c]] = np.asarray(res.results[c]["out"])
    return out, res


def kernel(**inputs):
    out, _ = run(inputs)
    return out



# revision 13
# speedup vs baseline: 1.0254x; 1.0254x over previous
"""Trainium2 Bass kernel for nn_BDLModel (gnn_message_passing).

Strategy (8 NeuronCores, SPMD):
  - Nodes sharded contiguously across cores (3750/core); within a core, nodes
    are assigned to 128-row dst tiles by balanced in-degree (LPT) to minimize
    the padded edge-chunk count CK; the output rows are inverse-permuted on
    the host.
  - Mean aggregation: the activation table is AllGathered to DRAM in fp8e4m3
    (values pre-scaled by 16; the 1/16 is folded into 1/deg), split into two
    half-tables so each collective fires as soon as its rows are written and
    overlaps compute. Per dst-tile, four dma_gathers (one per swdge queue,
    half-table split) pull all source rows ([128, CK, W] edge-major); per
    pair of 128-edge chunks a one-hot fp8 selection matrix S (iota vs
    per-edge dst-local ids) is the stationary operand of a DoubleRow fp8
    matmul: psum[d, :] += S^T @ Zgathered; 1/deg applied on PSUM evacuation.
  - SAGE MLPs run as fp8 DoubleRow matmuls (weights x32, activations x16,
    1/512 descale at evacuation); the MLP2 residual is added at evacuation.
  - Householder D=2 closed form: Q = [[c, s], [-s, c]], c=(a^2-1)/(1+a^2),
    s=2a/(1+a^2); only the 2::4 columns of enc_w2 are needed.
  - Self-z activations stay resident in SBUF; biases/residuals are preloaded
    into PSUM (Scalar/Vector) so matmul chains accumulate on top; PSUM
    evacuations and LayerNorm row-stats run on the Scalar engine.
"""

import math
import os
import sys
from dataclasses import dataclass

import numpy as np

for _p in ("/opt/trn_rl_repo", "/root/.axon_site/_ro/trn_rl_repo"):
    if os.path.isdir(_p) and _p not in sys.path:
        sys.path.insert(0, _p)

import ml_dtypes  # noqa: E402

BF16 = ml_dtypes.bfloat16

F8SCALE = 16.0


@dataclass(frozen=True)
class Cfg:
    N: int = 30000
    E: int = 480000
    HID: int = 256
    NB: int = 128
    D: int = 2
    NL: int = 2
    NSAGE: int = 5
    OUT: int = 5
    NC: int = 8
    EPS: float = 1e-5

    @property
    def SW(self):
        return self.D * self.D * self.NB

    @property
    def NLOC(self):
        return self.N // self.NC

    @property
    def NT(self):
        return (self.NLOC + 127) // 128

    @property
    def HLF(self):
        """Half-split row boundary for the two per-layer table collectives."""
        return self.NLOC // 2

    @property
    def HT(self):
        """First tile index whose rows complete the A half."""
        return (self.HLF + 127) // 128


CFG = Cfg()


# ---------------------------------------------------------------- host prep


W8SCALE = 32.0
F8NP = getattr(ml_dtypes, "float8_e4m3fn", None) or ml_dtypes.float8_e4m3


def _prep_rhs8(w):
    """fp8 weight panel, pre-scaled by W8SCALE; same [128, kc*M] layout."""
    k, m = w.shape
    kc = k // 128
    return np.ascontiguousarray(
        (w * W8SCALE).reshape(kc, 128, m).transpose(1, 0, 2).reshape(128, kc * m)
    ).astype(F8NP)


def _prep_rhs(w):
    """[K, M] -> [128, (K//128)*M] so slice kc -> [:, kc*M:(kc+1)*M] = W[kc]."""
    k, m = w.shape
    assert k % 128 == 0
    kc = k // 128
    return np.ascontiguousarray(
        w.reshape(kc, 128, m).transpose(1, 0, 2).reshape(128, kc * m)
    ).astype(BF16)


def _prep_bias(b):
    return np.ascontiguousarray(np.tile(np.asarray(b, np.float32).reshape(1, -1), (128, 1)))


def _balance_tiles(deg_local, nt):
    """LPT: assign nodes to nt tiles of <=128 nodes, balancing summed degree.
    Returns perm (tile-major node order)."""
    import heapq

    order = np.argsort(-deg_local, kind="stable")
    heap = [(0, t) for t in range(nt)]
    heapq.heapify(heap)
    counts = [0] * nt
    members = [[] for _ in range(nt)]
    for v in order:
        while True:
            load, t = heapq.heappop(heap)
            if counts[t] < 128:
                break
        members[t].append(v)
        counts[t] += 1
        if counts[t] < 128:
            heapq.heappush(heap, (load + int(deg_local[v]), t))
    return np.concatenate([np.asarray(m, np.int64) for m in members])


def _prep_edges(cfg: Cfg, edge_index):
    """Balanced tile assignment (tiles ordered by descending edge count so the
    shared program's per-iteration chunk maxima stay tight); edges partitioned
    by dst owner; exact per-tile chunk schedule."""
    src = np.asarray(edge_index[0], np.int64)
    dst = np.asarray(edge_index[1], np.int64)
    deg = np.bincount(dst, minlength=cfg.N).astype(np.float64)
    rdeg_full = (1.0 / np.maximum(deg, 1.0)).astype(np.float32) / F8SCALE

    NLOC, NT = cfg.NLOC, cfg.NT
    # per-core node order: LPT perm, then 128-blocks sorted by edge count
    # (partial block pinned last so rows_of(t) stays program-side).
    perms, poss = [], []
    for c in range(cfg.NC):
        dl = deg[c * NLOC : (c + 1) * NLOC]
        perm0 = _balance_tiles(dl, NT)
        pos0 = np.empty(NLOC, np.int64)
        pos0[perm0] = np.arange(NLOC)
        m = (dst >= c * NLOC) & (dst < (c + 1) * NLOC)
        blk = pos0[dst[m] - c * NLOC] // 128
        cnt = np.bincount(blk, minlength=NT)
        full = np.argsort(-cnt[: NT - 1], kind="stable")
        order = np.concatenate([full, [NT - 1]])
        perm = np.concatenate([perm0[b * 128 : b * 128 + 128] for b in order])
        pos = np.empty(NLOC, np.int64)
        pos[perm] = np.arange(NLOC)
        perms.append(perm)
        poss.append(pos)

    # table row of global node v: half-split layout. srcs with pos < HLF live
    # in table A at row c*HLF + pos; the rest in table B at c*HLF + (pos-HLF).
    HLF = cfg.HLF
    src_c = src // NLOC
    src_p = np.concatenate([poss[c][None, :] for c in range(cfg.NC)], axis=0)[
        src_c, src % NLOC
    ]
    in_a = src_p < HLF
    src_row = np.where(in_a, src_c * HLF + src_p, src_c * (NLOC - HLF) + src_p - HLF)
    assert src_row.max() < 32768

    per_core = []
    cnta_all = np.zeros((cfg.NC, NT), np.int64)
    cntb_all = np.zeros((cfg.NC, NT), np.int64)
    for c in range(cfg.NC):
        lo, hi = c * NLOC, (c + 1) * NLOC
        m = (dst >= lo) & (dst < hi)
        s_row = src_row[m]
        s_a = in_a[m]
        d_pos = poss[c][dst[m] - lo]
        # order by (dst tile, B-half flag) so each tile lists A edges then B
        order = np.lexsort((~s_a, d_pos // 128))
        s_row, d_pos, s_a = s_row[order], d_pos[order], s_a[order]
        tb = np.searchsorted(d_pos // 128, np.arange(NT + 1))
        for t in range(NT):
            seg = s_a[tb[t] : tb[t + 1]]
            cnta_all[c, t] = int(seg.sum())
            cntb_all[c, t] = len(seg) - cnta_all[c, t]
        per_core.append((s_row, d_pos, tb, cnta_all[c].copy(), cntb_all[c].copy()))

    # shared per-iteration chunk counts: max over cores, at least 1
    cka = tuple(max(1, int(math.ceil(cnta_all[:, t].max() / 128))) for t in range(NT))
    ckb = tuple(max(1, int(math.ceil(cntb_all[:, t].max() / 128))) for t in range(NT))
    toff = [0] * (NT + 1)
    for t in range(NT):
        toff[t + 1] = toff[t] + cka[t] + ckb[t]
    TC = toff[NT]

    outs = []
    for c in range(cfg.NC):
        s_row, d_pos, tb, cnta, cntb = per_core[c]
        idx16 = np.zeros((128, TC * 8), np.int16)
        ids = np.full((128, TC), 255.0, BF16)
        for t in range(NT):
            na, nb = int(cnta[t]), int(cntb[t])
            for (n, roff, coff) in ((na, tb[t], toff[t]), (nb, tb[t] + na, toff[t] + cka[t])):
                if n == 0:
                    continue
                i = np.arange(n)
                # gather order: unwrapped[i] = idx16[i%16, i//16] (replicated x8)
                col = coff * 8 + i // 16
                row = i % 16
                sv = s_row[roff : roff + n]
                for g in range(8):
                    idx16[row + 16 * g, col] = sv
                ids[i % 128, coff + i // 128] = (
                    d_pos[roff : roff + n] - t * 128
                ).astype(BF16)
        rdeg = np.ones((128, NT), np.float32) / F8SCALE
        rfull = rdeg_full[c * NLOC : (c + 1) * NLOC][perms[c]]
        for t in range(NT):
            r0 = t * 128
            nr = min(128, NLOC - r0)
            rdeg[:nr, t] = rfull[r0 : r0 + nr]
        outs.append(dict(idx16=idx16, ids=ids, rdeg=rdeg, rdeg16=rdeg * F8SCALE))
    return (cka, ckb), outs, perms


def _prep_inputs(cfg: Cfg, inputs):
    """Build the per-core in_maps. Returns ((CKA, CKB), in_maps, perms)."""
    f32 = np.float32
    f64 = np.float64
    x = np.asarray(inputs["x"], f32)
    CK, edge_outs, perms = _prep_edges(cfg, np.asarray(inputs["edge_index"]))

    g = lambda k: np.asarray(inputs[k], f32)

    # fold the struct-encoder output projection into the first encoder MLP
    so_w = np.asarray(inputs["se_out_w"], f64)
    so_b = np.asarray(inputs["se_out_b"], f64)
    w_f1, b_f1 = [], []
    for k in range(cfg.NL):
        e1w = np.asarray(inputs["enc_w1"], f64)[k]
        e1b = np.asarray(inputs["enc_b1"], f64)[k]
        w_f1.append((so_w @ e1w).astype(f32))
        b_f1.append((so_b @ e1w + e1b).astype(f32))

    shared = {
        "w_in_r": _prep_rhs(g("w_in")),
        "b_in_bc": _prep_bias(g("b_in")),
        "w_si_r": _prep_rhs(g("se_in_w")),
        "b_si_bc": _prep_bias(g("se_in_b")),
        "w_s1_r": np.concatenate([_prep_rhs8(g("sage_w1")[i]) for i in range(cfg.NSAGE)], axis=1),
        "b_s1_bc": np.concatenate(
            [_prep_bias(g("sage_b1")[i] * F8SCALE * W8SCALE) for i in range(cfg.NSAGE)], axis=1
        ).astype(BF16),
        "w_s2_r": np.concatenate([_prep_rhs8(g("sage_w2")[i]) for i in range(cfg.NSAGE)], axis=1),
        "b_s2_bc": np.concatenate(
            [_prep_bias(g("sage_b2")[i] * F8SCALE * W8SCALE) for i in range(cfg.NSAGE)], axis=1
        ).astype(BF16),
        "w_e1_r": np.concatenate([_prep_rhs(w_f1[k]) for k in range(cfg.NL)], axis=1),
        "b_e1_bc": np.concatenate([_prep_bias(b_f1[k]) for k in range(cfg.NL)], axis=1),
        "w_e2_r": np.concatenate(
            [_prep_rhs(np.ascontiguousarray(g("enc_w2")[k][:, 2::4])) for k in range(cfg.NL)], axis=1
        ),
        "b_e2_bc": np.concatenate([_prep_bias(g("enc_b2")[k][2::4]) for k in range(cfg.NL)], axis=1),
        "ln_g_bc": np.concatenate([_prep_bias(g("ln_g")[k]) for k in range(cfg.NL)], axis=1),
        "ln_b_bc": np.concatenate([_prep_bias(g("ln_b")[k]) for k in range(cfg.NL)], axis=1),
        "w_b1_r": np.concatenate([_prep_rhs(g("bdl_w1")[k]) for k in range(cfg.NL)], axis=1),
        "b_b1_bc": np.concatenate([_prep_bias(g("bdl_b1")[k]) for k in range(cfg.NL)], axis=1),
        "w_b2_r": np.concatenate([_prep_rhs(g("bdl_w2")[k]) for k in range(cfg.NL)], axis=1),
        "b_b2_bc": np.concatenate([_prep_bias(g("bdl_b2")[k]) for k in range(cfg.NL)], axis=1),
        "oln_g_bc": _prep_bias(g("out_ln_g")),
        "oln_b_bc": _prep_bias(g("out_ln_b")),
        "w_o_r": _prep_rhs(g("w_out")),
        "b_o_bc": _prep_bias(g("b_out")),
        "ident_b": np.eye(128, dtype=BF16),
        "iota_f": np.tile(np.arange(128), (128, 1)).astype(BF16),
    }

    in_maps = []
    for c in range(cfg.NC):
        m = dict(shared)
        m["x_c"] = np.ascontiguousarray(x[c * cfg.NLOC : (c + 1) * cfg.NLOC][perms[c]])
        m["idx16"] = edge_outs[c]["idx16"]
        m["ids_f"] = edge_outs[c]["ids"]
        m["rdeg"] = edge_outs[c]["rdeg"]
        m["rdeg16"] = edge_outs[c]["rdeg16"]
        in_maps.append(m)
    return CK, in_maps, perms


# ---------------------------------------------------------------- builder


def build_program(cfg: Cfg, cka: tuple, ckb: tuple, sage_bias_zero: bool = True):
    from concourse import bacc, mybir
    import concourse.tile as tile

    f32 = mybir.dt.float32
    bf16 = mybir.dt.bfloat16
    fp8 = mybir.dt.float8e4
    i16 = mybir.dt.int16
    ALU = mybir.AluOpType
    AX = mybir.AxisListType
    ACT = mybir.ActivationFunctionType

    NT, NLOC, HID, SW = cfg.NT, cfg.NLOC, cfg.HID, cfg.SW
    HLF, HT = cfg.HLF, cfg.HT
    toff = [0] * (NT + 1)
    for _t in range(NT):
        toff[_t + 1] = toff[_t] + cka[_t] + ckb[_t]
    TC = toff[NT]
    CKMAX = max(cka[_t] + ckb[_t] for _t in range(NT))

    nc = bacc.Bacc(
        "TRN2",
        target_bir_lowering=False,
        debug=False,
        enable_asserts=False,
        num_devices=cfg.NC,
        num_swdge_queues=4,
    )
    rg = [list(range(cfg.NC))]

    # ---- external I/O
    d_x = nc.dram_tensor("x_c", [NLOC, HID], f32, kind="ExternalInput").ap()
    d_idx = nc.dram_tensor("idx16", [128, TC * 8], i16, kind="ExternalInput").ap()
    d_ids = nc.dram_tensor("ids_f", [128, TC], bf16, kind="ExternalInput").ap()
    d_rdeg = nc.dram_tensor("rdeg", [128, NT], f32, kind="ExternalInput").ap()
    d_rdeg16 = nc.dram_tensor("rdeg16", [128, NT], f32, kind="ExternalInput").ap()

    def din(name, shape, dt):
        return nc.dram_tensor(name, shape, dt, kind="ExternalInput").ap()

    NS, NL = cfg.NSAGE, cfg.NL
    d_w_in = din("w_in_r", [128, 2 * HID], bf16)
    d_b_in = din("b_in_bc", [128, HID], f32)
    d_w_si = din("w_si_r", [128, 2 * SW], bf16)
    d_b_si = din("b_si_bc", [128, SW], f32)
    d_w_s1 = din("w_s1_r", [128, NS * 8 * SW], fp8)
    d_b_s1 = din("b_s1_bc", [128, NS * SW], bf16)
    d_w_s2 = din("w_s2_r", [128, NS * 4 * SW], fp8)
    d_b_s2 = din("b_s2_bc", [128, NS * SW], bf16)
    d_w_e1 = din("w_e1_r", [128, NL * 4 * SW], bf16)
    d_b_e1 = din("b_e1_bc", [128, NL * SW], f32)
    d_w_e2 = din("w_e2_r", [128, NL * 4 * 128], bf16)
    d_b_e2 = din("b_e2_bc", [128, NL * 128], f32)
    d_ln_g = din("ln_g_bc", [128, NL * HID], f32)
    d_ln_b = din("ln_b_bc", [128, NL * HID], f32)
    d_w_b1 = din("w_b1_r", [128, NL * 4 * HID], bf16)
    d_b_b1 = din("b_b1_bc", [128, NL * HID], f32)
    d_w_b2 = din("w_b2_r", [128, NL * 2 * HID], bf16)
    d_b_b2 = din("b_b2_bc", [128, NL * HID], f32)
    d_oln_g = din("oln_g_bc", [128, HID], f32)
    d_oln_b = din("oln_b_bc", [128, HID], f32)
    d_w_o = din("w_o_r", [128, 2 * cfg.OUT], bf16)
    d_b_o = din("b_o_bc", [128, cfg.OUT], f32)
    d_identb = din("ident_b", [128, 128], bf16)
    d_iota = din("iota_f", [128, 128], bf16)

    d_out = nc.dram_tensor("out", [NLOC, cfg.OUT], f32, kind="ExternalOutput").ap()

    with tile.TileContext(nc) as tc:
        from contextlib import ExitStack

        ctx = ExitStack()
        pers = ctx.enter_context(tc.tile_pool(name="pers", bufs=1))
        wout = ctx.enter_context(tc.tile_pool(name="wout", bufs=1))
        wsage = ctx.enter_context(tc.tile_pool(name="wsage", bufs=1))
        work = ctx.enter_context(tc.tile_pool(name="work", bufs=2))
        small = ctx.enter_context(tc.tile_pool(name="small", bufs=2))
        spool = ctx.enter_context(tc.tile_pool(name="spool", bufs=2))
        zgp = ctx.enter_context(tc.tile_pool(name="zgp", bufs=2))
        dram = ctx.enter_context(tc.tile_pool(name="dram", bufs=2, space="DRAM"))
        psum = ctx.enter_context(tc.tile_pool(name="psum", bufs=2, space="PSUM"))

        # ---- persistent SBUF residents
        identb = pers.tile([128, 128], bf16, name="identb")
        iota = pers.tile([128, 128], bf16, name="iota")
        rdeg = pers.tile([128, NT], f32, name="rdegs")
        rdeg16 = pers.tile([128, NT], f32, name="rdeg16s")
        ids = pers.tile([128, TC], bf16, name="idss")
        idx = pers.tile([128, TC * 8], i16, name="idxs")
        nc.sync.dma_start(out=identb[:], in_=d_identb[:])
        nc.sync.dma_start(out=iota[:], in_=d_iota[:])
        nc.sync.dma_start(out=rdeg[:], in_=d_rdeg[:])
        nc.sync.dma_start(out=rdeg16[:], in_=d_rdeg16[:])
        nc.sync.dma_start(out=ids[:], in_=d_ids[:])
        nc.sync.dma_start(out=idx[:], in_=d_idx[:])

        ones1 = pers.tile([1, 128], bf16, name="ones1")
        nc.gpsimd.memset(ones1[:], 1.0)
        h_t = [pers.tile([128, HID], f32, name=f"h{t}") for t in range(NT)]
        z_t = [pers.tile([128, SW], bf16, name=f"z{t}") for t in range(NT)]
        c_t = [pers.tile([128, 128], bf16, name=f"rc{t}") for t in range(NT)]
        s_t = [pers.tile([128, 128], bf16, name=f"rs{t}") for t in range(NT)]

        def load_w(pool, name, src, cols, dt):
            t = pool.tile([128, cols], dt, tag=name, name=name)
            nc.sync.dma_start(out=t[:], in_=src)
            return t

        def load_w3(pool, name, src, kchunks, m, dt):
            """Weight panel as a native 3-D tile so DoubleRow rhs slices match
            the zg access-pattern shape exactly."""
            t = pool.tile([128, kchunks, m], dt, tag=name, name=name)
            nc.sync.dma_start(out=t[:].rearrange("p a b -> p (a b)"), in_=src)
            return t

        def dram_tile(name, shape, dt, shared=False):
            return dram.tile(
                shape, dt, tag=name, name=name, addr_space="Shared" if shared else "Local"
            )

        def rows_of(t):
            return min(128, NLOC - t * 128)

        # ---------- helpers ----------
        def transpose_into(dst, src_ap, nchunks, out_scale=None):
            """dst[:, kc*128:(kc+1)*128] = src[:, kc*128:(kc+1)*128]^T (bf16).
            All chunks land in one PSUM bank (start zero-fills the region),
            evacuated with a single vector copy."""
            tp = psum.tile([128, 4 * 128], bf16, tag="tr", name="trb")
            for kc in range(nchunks):
                nc.tensor.matmul(
                    tp[:, kc * 128 : (kc + 1) * 128],
                    lhsT=src_ap[:, kc * 128 : (kc + 1) * 128],
                    rhs=identb[:],
                    is_transpose=True,
                    start=(kc == 0),
                    stop=(kc == nchunks - 1),
                )
            if out_scale is None:
                nc.vector.tensor_copy(out=dst[:, : nchunks * 128], in_=tp[:, : nchunks * 128])
            else:
                nc.scalar.mul(
                    out=dst[:, : nchunks * 128], in_=tp[:, : nchunks * 128], mul=out_scale
                )

        def mm_acc(ps_ap, lhsT_tile, rhs_tile, kcs, m, rhs_block, preloaded=False):
            """ps (+)= sum_kc lhsT[:, kc]^T @ rhs[:, kc-block] (node-major out)."""
            for kc in range(kcs):
                nc.tensor.matmul(
                    ps_ap,
                    lhsT=lhsT_tile[:, kc * 128 : (kc + 1) * 128],
                    rhs=rhs_tile[:, kc * rhs_block + m[0] : kc * rhs_block + m[1]],
                    start=(kc == 0 and not preloaded),
                    stop=(kc == kcs - 1),
                    skip_group_check=preloaded,
                )

        def mm_dr(ps_ap, lhsT8, rhs8_3d, kpairs, preloaded=False):
            """ps (+)= paired fp8 DoubleRow accumulation over 2*kpairs chunks.
            rhs8_3d is a [128, 2*kpairs, M] tile; native slices keep the
            dual-pump stream rate."""
            for p in range(kpairs):
                nc.tensor.matmul(
                    ps_ap,
                    lhsT=lhsT8[:, 2 * p * 128 : (2 * p + 2) * 128].rearrange(
                        "q (two m) -> q two m", two=2
                    ),
                    rhs=rhs8_3d[:, 2 * p : 2 * p + 2, :],
                    start=(p == 0 and not preloaded),
                    stop=(p == kpairs - 1),
                    perf_mode=mybir.MatmulPerfMode.DoubleRow,
                    skip_group_check=preloaded,
                )

        def emit_ln(h_ap, g_bc_ap, b_bc_ap, out_ap, w):
            """LayerNorm with row stats on the Vector engine (bn_stats) and a
            table-free Newton rsqrt (avoids Sqrt activation-table churn)."""
            i32 = mybir.dt.int32
            st6 = small.tile([128, 6], f32, tag="ln1", name="ln1")
            nc.vector.bn_stats(out=st6[:], in_=h_ap)
            mv = small.tile([128, 2], f32, tag="ln2", name="ln2")
            nc.vector.bn_aggr(out=mv[:], in_=st6[:])
            vm = small.tile([128, 1], f32, tag="ln3", name="ln3")
            nc.vector.tensor_scalar(vm[:], mv[:, 1:2], 1.0, cfg.EPS, ALU.mult, ALU.add)
            y0i = small.tile([128, 1], i32, tag="ln4", name="ln4")
            nc.vector.tensor_scalar(y0i[:], vm[:].bitcast(i32), 1, None, ALU.logical_shift_right)
            nc.vector.tensor_scalar(y0i[:], y0i[:], -1, 0x5F3759DF, ALU.mult, ALU.add)
            rs = small.tile([128, 1], f32, tag="ln5", name="ln5")
            vy = small.tile([128, 1], f32, tag="ln6", name="ln6")
            ycur = y0i[:].bitcast(f32)
            for _ in range(2):
                nc.vector.tensor_tensor(out=vy[:], in0=vm[:], in1=ycur, op=ALU.mult)
                nc.vector.tensor_tensor(out=vy[:], in0=vy[:], in1=ycur, op=ALU.mult)
                nc.vector.tensor_scalar(vy[:], vy[:], -0.5, 1.5, ALU.mult, ALU.add)
                nc.vector.tensor_tensor(out=rs[:], in0=ycur, in1=vy[:], op=ALU.mult)
                ycur = rs[:]
            nmrs = small.tile([128, 1], f32, tag="ln7", name="ln7")
            nc.vector.scalar_tensor_tensor(
                out=nmrs[:], in0=mv[:, 0:1], scalar=-1.0, in1=rs[:],
                op0=ALU.mult, op1=ALU.mult,
            )
            hn0 = work.tile([128, w], bf16, tag="lnhn0", name="lnhn0")
            nc.scalar.activation(
                out=hn0[:], in_=h_ap, func=ACT.Identity, bias=nmrs[:], scale=rs[:]
            )
            hn1 = work.tile([128, w], bf16, tag="lnhn1", name="lnhn1")
            nc.vector.tensor_tensor(out=hn1[:], in0=hn0[:], in1=g_bc_ap, op=ALU.mult)
            nc.vector.tensor_tensor(out=out_ap, in0=hn1[:], in1=b_bc_ap, op=ALU.add)

        def emit_agg(tabA_ap, tabB_ap, t, width, ps_ap):
            """Gather + one-hot matmul segment sum for dst tile t into psum.
            A-half chunks (queues 0/1) read tabA; B-half (queues 2/3) tabB.
            Exact per-tile chunk counts."""
            CKA_t, CKB_t = cka[t], ckb[t]
            CK_t = CKA_t + CKB_t
            T0 = toff[t]
            zg = zgp.tile([128, CKMAX, width], fp8, tag="zg", name="zg", bufs=5)
            ca2, cb2 = (CKA_t + 1) // 2, (CKB_t + 1) // 2
            calls = (
                (0, 0, ca2, tabA_ap),
                (1, ca2, CKA_t, tabA_ap),
                (2, CKA_t, CKA_t + cb2, tabB_ap),
                (3, CKA_t + cb2, CK_t, tabB_ap),
            )
            for qi, c0, c1, tab in calls:
                if c1 > c0:
                    nc.gpsimd.dma_gather(
                        out_ap=zg[:, c0:c1, :],
                        in_ap=tab,
                        idxs_ap=idx[:, T0 * 8 + c0 * 8 : T0 * 8 + c1 * 8],
                        num_idxs=(c1 - c0) * 128,
                        num_idxs_reg=(c1 - c0) * 128,
                        elem_size=width,
                        single_packet=True,
                        queue_num=qi,
                    )
            Sall = spool.tile([128, CKMAX * 128], fp8, tag="S", name="S", bufs=4)
            iota_bc = iota[:].rearrange("p (o f) -> p o f", o=1).to_broadcast([128, CK_t, 128])
            ids_bc = (
                ids[:, T0 : T0 + CK_t]
                .rearrange("p (c o) -> p c o", o=1)
                .to_broadcast([128, CK_t, 128])
            )
            nc.vector.tensor_tensor(
                out=Sall[:, : CK_t * 128], in0=iota_bc, in1=ids_bc, op=ALU.is_equal
            )
            npair = CK_t // 2
            for p in range(npair):
                nc.tensor.matmul(
                    ps_ap,
                    lhsT=Sall[:, 2 * p * 128 : (2 * p + 2) * 128].rearrange(
                        "q (two m) -> q two m", two=2
                    ),
                    rhs=zg[:, 2 * p : 2 * p + 2, :],
                    start=(p == 0),
                    stop=(p == npair - 1 and CK_t % 2 == 0),
                    perf_mode=mybir.MatmulPerfMode.DoubleRow,
                )
            if CK_t % 2:
                nc.tensor.matmul(
                    ps_ap,
                    lhsT=Sall[:, (CK_t - 1) * 128 : CK_t * 128],
                    rhs=zg[:, CK_t - 1, :],
                    start=(CK_t == 1),
                    stop=True,
                )

        def fire_halves(t, loc_ap, tabA_ap, tabB_ap):
            """Issue the half-table AllGathers as their rows complete."""
            if t == HT - 1:
                nc.gpsimd.collective_compute(
                    "AllGather", ALU.bypass, replica_groups=rg,
                    ins=[loc_ap[0:HLF, :]], outs=[tabA_ap],
                )
            elif t == NT - 1:
                nc.gpsimd.collective_compute(
                    "AllGather", ALU.bypass, replica_groups=rg,
                    ins=[loc_ap[HLF:NLOC, :]], outs=[tabB_ap],
                )

        def emit_z0(t, w_si_sb, b_si_sb, loc8_ap, tabA_ap, tabB_ap, hb16):
            """z0 = gelu(h@w_si+b_si) -> z_t[t] (bf16) + fp8 table row write."""
            nr = rows_of(t)
            hT = work.tile([128, 2 * 128], bf16, tag="hT", name="hT")
            transpose_into(hT, hb16[:], 2)
            zp = psum.tile([128, SW], f32, tag="mlp", name="zp", bufs=3)
            nc.scalar.copy(out=zp[:], in_=b_si_sb[:])
            mm_acc(zp[:], hT, w_si_sb, 2, (0, SW), SW, preloaded=True)
            nc.scalar.activation(out=z_t[t][:], in_=zp[:], func=ACT.Gelu)
            z8 = work.tile([128, SW], fp8, tag="z8", name="z8", bufs=3)
            nc.vector.tensor_scalar(z8[:], z_t[t][:], F8SCALE, None, ALU.mult)
            nc.sync.dma_start(out=loc8_ap[t * 128 : t * 128 + nr, :], in_=z8[:nr, :])
            fire_halves(t, loc8_ap, tabA_ap, tabB_ap)

        # ================= phase 0: h0 = gelu(x @ w_in + b_in), z0 =================
        w_in_sb = load_w(wout, "w_in", d_w_in[:], 2 * HID, bf16)
        b_in_sb = load_w(wout, "b_in", d_b_in[:], HID, f32)
        w_si_sb = load_w(wout, "w_si", d_w_si[:], 2 * SW, bf16)
        b_si_sb = load_w(wout, "b_si", d_b_si[:], SW, f32)

        loc8 = dram_tile("loc8", [NLOC, SW], fp8)
        tabA = dram_tile("tabA", [cfg.NC * HLF, SW], fp8, shared=True)
        tabB = dram_tile("tabB", [cfg.NC * (NLOC - HLF), SW], fp8, shared=True)
        for t in range(NT):
            nr = rows_of(t)
            xt = work.tile([128, HID], f32, tag="lnjunk", name="xt")
            if nr < 128:
                nc.gpsimd.memset(xt[:], 0.0)
            nc.sync.dma_start(out=xt[:nr, :], in_=d_x[t * 128 : t * 128 + nr, :])
            xb = work.tile([128, HID], bf16, tag="hb16", name="xb")
            nc.vector.tensor_copy(out=xb[:], in_=xt[:])
            xT = work.tile([128, 2 * 128], bf16, tag="xT", name="xT")
            transpose_into(xT, xb[:], 2)
            hp = psum.tile([128, HID], f32, tag="mlp", name="hp", bufs=3)
            nc.scalar.copy(out=hp[:], in_=b_in_sb[:])
            mm_acc(hp[:], xT, w_in_sb, 2, (0, HID), HID, preloaded=True)
            nc.scalar.activation(out=h_t[t][:], in_=hp[:], func=ACT.Gelu)
            hb16 = work.tile([128, HID], bf16, tag="hb16", name="hb16")
            nc.vector.tensor_copy(out=hb16[:], in_=h_t[t][:])
            emit_z0(t, w_si_sb, b_si_sb, loc8, tabA, tabB, hb16)

        # ================= outer layers =================
        for k in range(NL):
            # ---- per-outer weights
            w_e1_sb = load_w(wout, "w_e1", d_w_e1[:, k * 4 * SW : (k + 1) * 4 * SW], 4 * SW, bf16)
            b_e1_sb = load_w(wout, "b_e1", d_b_e1[:, k * SW : (k + 1) * SW], SW, f32)
            w_e2_sb = load_w(wout, "w_e2", d_w_e2[:, k * 4 * 128 : (k + 1) * 4 * 128], 4 * 128, bf16)
            b_e2_sb = load_w(wout, "b_e2", d_b_e2[:, k * 128 : (k + 1) * 128], 128, f32)
            ln_g_sb = load_w(wout, "ln_g", d_ln_g[:, k * HID : (k + 1) * HID], HID, f32)
            ln_b_sb = load_w(wout, "ln_b", d_ln_b[:, k * HID : (k + 1) * HID], HID, f32)
            w_b1_sb = load_w(wout, "w_b1", d_w_b1[:, k * 4 * HID : (k + 1) * 4 * HID], 4 * HID, bf16)
            b_b1_sb = load_w(wout, "b_b1", d_b_b1[:, k * HID : (k + 1) * HID], HID, f32)
            w_b2_sb = load_w(wout, "w_b2", d_w_b2[:, k * 2 * HID : (k + 1) * 2 * HID], 2 * HID, bf16)
            b_b2_sb = load_w(wout, "b_b2", d_b_b2[:, k * HID : (k + 1) * HID], HID, f32)

            tabA_prev, tabB_prev = tabA, tabB
            locy8 = dram_tile("locy8", [NLOC, HID], fp8)
            ytabA = dram_tile("ytabA", [cfg.NC * HLF, HID], fp8, shared=True)
            ytabB = dram_tile("ytabB", [cfg.NC * (NLOC - HLF), HID], fp8, shared=True)

            # ---- SAGE layers
            for i in range(cfg.NSAGE):
                w1_sb = load_w3(wsage, "w1", d_w_s1[:, i * 8 * SW : (i + 1) * 8 * SW], 8, SW, fp8)
                b1_sb = load_w(wsage, "b1", d_b_s1[:, i * SW : (i + 1) * SW], SW, bf16)
                w2_sb = load_w3(wsage, "w2", d_w_s2[:, i * 4 * SW : (i + 1) * 4 * SW], 4, SW, fp8)
                b2_sb = load_w(wsage, "b2", d_b_s2[:, i * SW : (i + 1) * SW], SW, bf16)
                last = i == cfg.NSAGE - 1
                if not last:
                    loc8_cur = dram_tile("loc8", [NLOC, SW], fp8)
                    tabA_cur = dram_tile("tabA", [cfg.NC * HLF, SW], fp8, shared=True)
                    tabB_cur = dram_tile("tabB", [cfg.NC * (NLOC - HLF), SW], fp8, shared=True)

                for t in range(NT):
                    nr = rows_of(t)
                    # aggregation from previous table
                    aps = psum.tile([128, SW], f32, tag="agg", name="aps", bufs=3)
                    emit_agg(tabA_prev[:], tabB_prev[:], t, SW, aps[:])
                    m_sb = work.tile([128, SW], bf16, tag="msb", name="msb", bufs=3)
                    nc.scalar.mul(out=m_sb[:], in_=aps[:], mul=rdeg16[:, t : t + 1])
                    # zc^T = [z*16 | m*16]^T in fp8 (m_sb already x16 scaled)
                    zcT8 = work.tile([128, 8 * 128], fp8, tag="zcT", name="zcT8", bufs=3)
                    transpose_into(zcT8[:, : 4 * 128], z_t[t][:], 4, out_scale=F8SCALE)
                    transpose_into(zcT8[:, 4 * 128 : 8 * 128], m_sb[:], 4, out_scale=1.0)
                    # MLP1 (fp8 DoubleRow; psum holds 512x the true values)
                    p1p = psum.tile([128, SW], f32, tag="mlp", name="p1p", bufs=3)
                    if not sage_bias_zero:
                        nc.tensor.matmul(
                            p1p[:], lhsT=ones1[0:1, :], rhs=b1_sb[0:1, :], start=True, stop=False
                        )
                    mm_dr(p1p[:], zcT8, w1_sb, 4, preloaded=not sage_bias_zero)
                    p1 = work.tile([128, SW], bf16, tag="p1", name="p1", bufs=3)
                    nc.scalar.activation(out=p1[:], in_=p1p[:], func=ACT.Gelu, scale=1.0 / 512.0)
                    p1T8 = work.tile([128, 4 * 128], fp8, tag="p1T", name="p1T8", bufs=3)
                    transpose_into(p1T8, p1[:], 4, out_scale=F8SCALE)
                    # MLP2 (psum holds 512x); residual added at evacuation
                    p2p = psum.tile([128, SW], f32, tag="mlp", name="p2p", bufs=3)
                    if not sage_bias_zero:
                        nc.tensor.matmul(
                            p2p[:], lhsT=ones1[0:1, :], rhs=b2_sb[0:1, :], start=True, stop=False
                        )
                    mm_dr(p2p[:], p1T8, w2_sb, 2, preloaded=not sage_bias_zero)
                    nc.vector.scalar_tensor_tensor(
                        out=z_t[t][:], in0=p2p[:], scalar=1.0 / 512.0, in1=z_t[t][:],
                        op0=ALU.mult, op1=ALU.add,
                    )
                    if not last:
                        z8 = work.tile([128, SW], fp8, tag="z8", name="z8", bufs=3)
                        nc.vector.tensor_scalar(z8[:], z_t[t][:], F8SCALE, None, ALU.mult)
                        nc.sync.dma_start(
                            out=loc8_cur[t * 128 : t * 128 + nr, :], in_=z8[:nr, :]
                        )
                        fire_halves(t, loc8_cur, tabA_cur, tabB_cur)
                        continue

                    # ---- fused: enc path -> rotation coefs; LN(h) -> y -> locy8
                    z5T = work.tile([128, 4 * 128], bf16, tag="p1T", name="z5T", bufs=3)
                    transpose_into(z5T, z_t[t][:], 4)
                    gp = psum.tile([128, SW], f32, tag="mlp", name="gp", bufs=3)
                    nc.scalar.copy(out=gp[:], in_=b_e1_sb[:])
                    mm_acc(gp[:], z5T, w_e1_sb, 4, (0, SW), SW, preloaded=True)
                    gact = work.tile([128, SW], bf16, tag="p1", name="gact", bufs=3)
                    nc.scalar.activation(out=gact[:], in_=gp[:], func=ACT.Gelu)
                    gT = work.tile([128, 4 * 128], bf16, tag="p1T", name="gT", bufs=3)
                    transpose_into(gT, gact[:], 4)
                    ap_ = psum.tile([128, 128], f32, tag="agg", name="ap_", bufs=3)
                    nc.scalar.copy(out=ap_[:], in_=b_e2_sb[:])
                    mm_acc(ap_[:], gT, w_e2_sb, 4, (0, 128), 128, preloaded=True)
                    a_sb = work.tile([128, 128], f32, tag="a0", name="a_sb")
                    nc.scalar.copy(out=a_sb[:], in_=ap_[:])
                    a2 = work.tile([128, 128], f32, tag="a1", name="a2")
                    nc.vector.tensor_tensor(out=a2[:], in0=a_sb[:], in1=a_sb[:], op=ALU.mult)
                    rinv = work.tile([128, 128], f32, tag="a2t", name="rinv")
                    nc.vector.tensor_scalar(rinv[:], a2[:], 1.0, None, ALU.add)
                    nc.vector.reciprocal(out=rinv[:], in_=rinv[:])
                    nc.vector.tensor_scalar(a2[:], a2[:], -1.0, None, ALU.add)
                    nc.vector.tensor_tensor(out=c_t[t][:], in0=a2[:], in1=rinv[:], op=ALU.mult)
                    nc.vector.tensor_scalar(a_sb[:], a_sb[:], 2.0, None, ALU.mult)
                    nc.vector.tensor_tensor(out=s_t[t][:], in0=a_sb[:], in1=rinv[:], op=ALU.mult)

                    # LN(h) -> hn; y = rot(hn); y8 = y*16 fp8
                    hn = work.tile([128, HID], bf16, tag="hn", name="hn")
                    emit_ln(h_t[t][:], ln_g_sb[:], ln_b_sb[:], hn[:], HID)
                    hn_ev = hn[:, 0:HID:2]
                    hn_od = hn[:, 1:HID:2]
                    y = work.tile([128, HID], bf16, tag="y", name="y")
                    t0 = work.tile([128, 128], bf16, tag="r0", name="t0")
                    t1 = work.tile([128, 128], bf16, tag="r1", name="t1")
                    nc.vector.tensor_tensor(out=t0[:], in0=c_t[t][:], in1=hn_ev, op=ALU.mult)
                    nc.vector.tensor_tensor(out=t1[:], in0=s_t[t][:], in1=hn_od, op=ALU.mult)
                    nc.vector.tensor_tensor(out=y[:, 0:HID:2], in0=t0[:], in1=t1[:], op=ALU.add)
                    nc.vector.tensor_tensor(out=t0[:], in0=c_t[t][:], in1=hn_od, op=ALU.mult)
                    nc.vector.tensor_tensor(out=t1[:], in0=s_t[t][:], in1=hn_ev, op=ALU.mult)
                    nc.vector.tensor_tensor(
                        out=y[:, 1:HID:2], in0=t0[:], in1=t1[:], op=ALU.subtract
                    )
                    y8 = work.tile([128, HID], fp8, tag="y8", name="y8")
                    nc.vector.tensor_scalar(y8[:], y[:], F8SCALE, None, ALU.mult)
                    nc.sync.dma_start(
                        out=locy8[t * 128 : t * 128 + nr, :], in_=y8[:nr, :]
                    )
                    fire_halves(t, locy8, ytabA, ytabB)

                if not last:
                    tabA_prev, tabB_prev = tabA_cur, tabB_cur
                    loc8 = loc8_cur

            # ---- BDL message + MLP, h update (+ fused z0 of next layer / output)
            if k + 1 < NL:
                loc8_nxt = dram_tile("loc8", [NLOC, SW], fp8)
                tabA_nxt = dram_tile("tabA", [cfg.NC * HLF, SW], fp8, shared=True)
                tabB_nxt = dram_tile("tabB", [cfg.NC * (NLOC - HLF), SW], fp8, shared=True)
            else:
                oln_g_sb = load_w(wout, "oln_g", d_oln_g[:], HID, f32)
                oln_b_sb = load_w(wout, "oln_b", d_oln_b[:], HID, f32)
                w_o_sb = load_w(wout, "w_o", d_w_o[:], 2 * cfg.OUT, bf16)
                b_o_sb = load_w(wout, "b_o", d_b_o[:], cfg.OUT, f32)

            for t in range(NT):
                nr = rows_of(t)
                yps = psum.tile([128, HID], f32, tag="agg", name="yps", bufs=3)
                emit_agg(ytabA[:], ytabB[:], t, HID, yps[:])
                # hn first (independent of the aggregate)
                hn = work.tile([128, HID], bf16, tag="hn", name="hnb")
                emit_ln(h_t[t][:], ln_g_sb[:], ln_b_sb[:], hn[:], HID)
                ga = work.tile([128, HID], bf16, tag="ga", name="ga")
                nc.vector.tensor_scalar(ga[:], yps[:], rdeg[:, t : t + 1], None, ALU.mult)
                g_ev = ga[:, 0:HID:2]
                g_od = ga[:, 1:HID:2]
                msg = work.tile([128, HID], bf16, tag="msg", name="msg")
                t0 = work.tile([128, 128], bf16, tag="r0", name="t0b")
                t1 = work.tile([128, 128], bf16, tag="r1", name="t1b")
                nc.vector.tensor_tensor(out=t0[:], in0=c_t[t][:], in1=g_ev, op=ALU.mult)
                nc.vector.tensor_tensor(out=t1[:], in0=s_t[t][:], in1=g_od, op=ALU.mult)
                nc.vector.tensor_tensor(
                    out=msg[:, 0:HID:2], in0=t0[:], in1=t1[:], op=ALU.subtract
                )
                nc.vector.tensor_tensor(out=t0[:], in0=s_t[t][:], in1=g_ev, op=ALU.mult)
                nc.vector.tensor_tensor(out=t1[:], in0=c_t[t][:], in1=g_od, op=ALU.mult)
                nc.vector.tensor_tensor(
                    out=msg[:, 1:HID:2], in0=t0[:], in1=t1[:], op=ALU.add
                )
                hcT = work.tile([128, 4 * 128], bf16, tag="hcT", name="hcT")
                tp4 = psum.tile([128, 4 * 128], bf16, tag="tr", name="trh")
                for j, srcap in enumerate([hn[:, 0:128], hn[:, 128:256], msg[:, 0:128], msg[:, 128:256]]):
                    nc.tensor.matmul(
                        tp4[:, j * 128 : (j + 1) * 128], lhsT=srcap, rhs=identb[:],
                        is_transpose=True, start=(j == 0), stop=(j == 3),
                    )
                nc.vector.tensor_copy(out=hcT[:], in_=tp4[:])
                bp = psum.tile([128, HID], f32, tag="mlp", name="bp", bufs=3)
                nc.scalar.copy(out=bp[:], in_=b_b1_sb[:])
                mm_acc(bp[:], hcT, w_b1_sb, 4, (0, HID), HID, preloaded=True)
                tb = work.tile([128, HID], bf16, tag="tb", name="tb")
                nc.scalar.activation(out=tb[:], in_=bp[:], func=ACT.Gelu)
                tbT = work.tile([128, 2 * 128], bf16, tag="tbT", name="tbT")
                transpose_into(tbT, tb[:], 2)
                b2p = psum.tile([128, HID], f32, tag="mlp", name="b2p", bufs=3)
                nc.vector.tensor_tensor(out=b2p[:], in0=h_t[t][:], in1=b_b2_sb[:], op=ALU.add)
                mm_acc(b2p[:], tbT, w_b2_sb, 2, (0, HID), HID, preloaded=True)
                nc.vector.tensor_copy(out=h_t[t][:], in_=b2p[:])

                if k + 1 < NL:
                    hb16 = work.tile([128, HID], bf16, tag="hb16", name="hb16b")
                    nc.vector.tensor_copy(out=hb16[:], in_=h_t[t][:])
                    emit_z0(t, w_si_sb, b_si_sb, loc8_nxt, tabA_nxt, tabB_nxt, hb16)
                else:
                    hnf = work.tile([128, HID], bf16, tag="hn", name="hnf")
                    emit_ln(h_t[t][:], oln_g_sb[:], oln_b_sb[:], hnf[:], HID)
                    hnfT = work.tile([128, 2 * 128], bf16, tag="tbT", name="hnfT")
                    transpose_into(hnfT, hnf[:], 2)
                    op_ = psum.tile([128, cfg.OUT], f32, tag="mlp", name="op_", bufs=3)
                    nc.scalar.copy(out=op_[:], in_=b_o_sb[:])
                    mm_acc(op_[:], hnfT, w_o_sb, 2, (0, cfg.OUT), cfg.OUT, preloaded=True)
                    ot = work.tile([128, cfg.OUT], f32, tag="ot", name="ot")
                    nc.scalar.copy(out=ot[:], in_=op_[:])
                    nc.sync.dma_start(out=d_out[t * 128 : t * 128 + nr, :], in_=ot[:nr, :])

            if k + 1 < NL:
                tabA, tabB = tabA_nxt, tabB_nxt
                loc8 = loc8_nxt

        ctx.close()

    nc.compile()
    return nc


# ---------------------------------------------------------------- runner

_CACHE = {}


def _get_program(cfg: Cfg, cka: tuple, ckb: tuple, sage_bias_zero: bool):
    key = (cfg, cka, ckb, sage_bias_zero)
    if key not in _CACHE:
        _CACHE[key] = build_program(cfg, cka, ckb, sage_bias_zero)
    return _CACHE[key]


def run(inputs, cfg: Cfg = CFG, trace: bool = False):
    from concourse import bass_utils

    (cka, ckb), in_maps, perms = _prep_inputs(cfg, inputs)
    sbz = not (
        np.asarray(inputs["sage_b1"]).any() or np.asarray(inputs["sage_b2"]).any()
    )
    nc = _get_program(cfg, cka, ckb, sbz)
    res = bass_utils.run_bass_kernel_spmd(
        nc, in_maps, core_ids=list(range(cfg.NC)), trace=trace
    )
    out = np.empty((cfg.N, cfg.OUT), np.float32)
    for c in range(cfg.NC):
        out[c * cfg.NLOC + perms[c]] = np.asarray(res.results[c]["out"])
    return out, res


def kernel(**inputs):
    out, _ = run(inputs)
    return out

